# revision 28
# baseline (speedup 1.0000x reference)
"""ChebConv2D (K1=K2=3) Trainium2 Bass kernel.

Data-parallel over batch (B=8) across 8 NeuronCores; per core the whole
per-batch computation runs on-chip.

Math (per batch, x: [N, N, C], N=200, C=32, OUT=64):
    out = U_0 + R_L(U_1) + R_{L^2}(U_2) + bias
    U_j = sum_i (A^i x) @ W'_ij^T      (Chebyshev folded into W' on host)

v6: full f16, PE-stream-centric rewrite.
 - 128/72 k-tile splits on every 200-length contraction (n1 for S1, TT
   mem rows for S2, e for S3) so most stationaries are 128 columns.
 - S1 scatters in supers of 8 chunks (3200B descriptors); S2 half-0
   starts after 4 supers.
 - PSUM evacuated in multi-bank tiles (bank-aligned 256-f32 slots so
   matmul outputs never cross a 2KB bank); evac only on vector+scalar
   (gpsimd has no PSUM port), all DMAs issued from sync/gpsimd.
 - PE warmed from ~0.7us via memset-backed dummy matmuls (no DMA dep)
   so the HAM clock-gate opens before the real stream begins.
 - Output stored [n2_mem, n1, o] f16 per 2-block groups on alternating
   queues; host unpermutes/transposes/upcasts.
"""

import numpy as np

import concourse.bass as bass
import concourse.mybir as mybir
from concourse import bacc
import concourse.tile as tile
from concourse import bass_utils

N = 200
C = 32
OUT = 64
B = 8
F32 = mybir.dt.float32
F16 = mybir.dt.float16
MIXN = 192
KT0, KT1 = 128, 72      # row split for all 200-length contractions
SUP = 8                 # S1 chunks per scatter super
NSUP = 7                # ceil(50/8); last super has 2 chunks
XCH = 50                # whole x in one load pair: 12.8KB descriptors
NXG = 1
BLK = 8
NBLK = N // BLK
G0 = 4                  # n1 per S2h0 psum tile (2 per bank x 2 banks)
NG0 = N // G0
# output store groups (blocks per DMA): big early, small at the tail
OGS = [2] * 12 + [1]
OGI = [g for g, og in enumerate(OGS) for _ in range(og)]   # blk -> group
OGO = [i for og in OGS for i in range(og)]                 # blk -> pos
OGMAX = max(OGS)


def build_program():
    nc = bacc.Bacc("TRN2")

    x_d = nc.dram_tensor("x", [N, N * C], F16, kind="ExternalInput")
    xt_d = nc.dram_tensor("xt", [C, N * N], F16, kind="ExternalInput")
    # kb packs G tiles (400+400), lt tiles (4x200), ws (192) -> one DMA
    kb_d = nc.dram_tensor("kb", [KT0, 1792], F16, kind="ExternalInput")
    ones_d = nc.dram_tensor("ones", [1, N * N], F16, kind="ExternalInput")
    # out is [n2_mem, n1, o]; host unpermutes + transposes back
    out_d = nc.dram_tensor("out", [N, N, OUT], F16, kind="ExternalOutput")

    with tile.TileContext(nc) as tc:
        with (
            tc.tile_pool(name="const", bufs=1) as constp,
            tc.tile_pool(name="tt", bufs=1) as ttp,
            tc.tile_pool(name="u0", bufs=1) as u0p,
        ):
            TT = ttp.tile([3 * C + 1, N * N], F16, tag="TT")
            TT3 = TT[:].rearrange("p (a b) -> p a b", b=N)

            # U half 0 (mem rows 0:128) for all n1: [a, n1*192 + (j,o)]
            UC0 = u0p.tile([KT0, N * MIXN], F16, tag="UC0")
            UC03 = UC0[:].rearrange("p (n f) -> p n f", f=MIXN)
            # same storage viewed as (n1-pair, q, f) for 4D psum evacs
            UC04 = UC0[:].rearrange("p (nb q f) -> p nb q f", q=2, f=MIXN)

            with (
                tc.tile_pool(name="xa", bufs=1) as xap,
                tc.tile_pool(name="sg", bufs=2) as sgp,
                tc.tile_pool(name="uc", bufs=2) as ucp,
                tc.tile_pool(name="ob", bufs=1) as obp,
            ):
                # --- tiny warmup tile: memset (no DMA dep), then dummy
                # matmuls keep the PE activity monitor busy so the clock
                # un-throttles to 2.4 GHz before the real stream starts
                du = constp.tile([1, 512], F16, tag="du")
                nc.vector.memset(du[:], 0.0)
                psap_cm = tc.tile_pool(name="psA", bufs=4, space="PSUM")
                psap = psap_cm.__enter__()
                psd_cm = tc.tile_pool(name="psD", bufs=1, space="PSUM")
                psdp = psd_cm.__enter__()
                pdum = psdp.tile([128, 512], F32, tag="pdum")

                def pe_warm(n):
                    for _ in range(n):
                        nc.tensor.matmul(pdum[:], du[0:1, 0:128], du[:],
                                         start=True, stop=True)

                pe_warm(2)

                # --- input DMAs: x group 0 alone on sync, kb G-half on
                # gpsimd, so the first S1 matmul's deps land in parallel
                kb = constp.tile([KT0, 1792], F16, tag="kb")
                xq = {}

                def x_load(g):
                    # the whole x in one DMA pair: 12.8KB per-partition
                    # runs stream near bus rate on all engines, done by
                    # ~15us; no ring, no prefetch, no feed stalls
                    for t in range(2):
                        r0, r1 = (0, KT0) if t == 0 else (KT0, N)
                        xm = xap.tile([r1 - r0, XCH * 128], F16,
                                      tag=f"xm{t}", name=f"xm{t}_{g}")
                        nc.sync.dma_start(xm[:], x_d[r0:r1, :])
                        xq[(t, g)] = xm

                nc.sync.dma_start(kb[:, 0:800], kb_d[:, 0:800])
                x_load(0)
                nc.gpsimd.dma_start(kb[:, 800:1792], kb_d[:, 800:1792])
                nc.gpsimd.dma_start(TT[0:C, 0:KT0 * N], xt_d[:, 0:KT0 * N])
                nc.gpsimd.dma_start(TT[96:97, :], ones_d[:, :])

                g_t = [kb[0:KT0, 0:400], kb[0:KT1, 400:800]]
                lt_t = {(1, 0): kb[0:KT0, 800:1000],
                        (1, 1): kb[0:KT1, 1000:1200],
                        (2, 0): kb[0:KT0, 1200:1400],
                        (2, 1): kb[0:KT1, 1400:1600]}
                ws = kb[0:97, 1600:1792]

                pe_warm(6)

                psu0_cm = None
                psup0 = None

                def s1_super(s):
                    k0 = s * SUP
                    nk = min(SUP, 50 - k0)
                    # sc free layout: (i 2, k SUP, e N)
                    sc = sgp.tile([128, SUP * 2 * N], F16, tag="sc",
                                  name=f"sc_{s}")
                    sc4 = sc[:].rearrange("p (i k e) -> p i k e", i=2, e=N)
                    for k in range(nk):
                        m = k0 + k
                        psa = psap.tile([128, 2 * N], F32, tag="psa")
                        for t in range(2):
                            xm = xq[(t, m // XCH)]
                            mm = m % XCH
                            lhsT = xm[:, mm * 128:(mm + 1) * 128]
                            nc.tensor.matmul(psa[:], lhsT, g_t[t],
                                             start=(t == 0), stop=(t == 1))
                        dst = sc4[:, :, k, :]
                        src = psa[:].rearrange("p (i e) -> p i e", e=N)
                        if k % 2 == 0:
                            nc.vector.tensor_copy(dst, src)
                        else:
                            nc.scalar.copy(dst, src)
                    # scatter: sc[(c r), (i k e)] -> TT[(i c), 32s+8r+k, e]
                    # supers 0-3 gate S2h0: i=0 on sync, i=1 on gpsimd.
                    # Their queue-blocking waits also pace the prefetches
                    # behind them, keeping the queues shallow (round-robin
                    # completion punishes deep queues). Supers 4-6 feed
                    # phase 2 only -> gpsimd.
                    for i in range(2):
                        if s < 4:
                            eng = nc.sync if i == 0 else nc.gpsimd
                        else:
                            eng = nc.gpsimd
                        src = sc[:, i * SUP * N:i * SUP * N + nk * N]
                        dst = TT3[(1 + i) * C:(2 + i) * C,
                                  4 * SUP * s:4 * SUP * s + 4 * nk, :]
                        eng.dma_start(dst, src)
                    if s == 4:
                        nc.gpsimd.dma_start(TT[0:C, KT0 * N:N * N],
                                            xt_d[:, KT0 * N:N * N])

                def s2h0_grp(g):
                    # 4 n1 per 2-bank psum tile: slot layout (nb 2, q 2,
                    # s 256) so each 192-col matmul stays inside a bank
                    n1_0 = g * G0
                    psu = psup0.tile([KT0, 2 * 512], F32, tag="psu0",
                                     name=f"psu0_{g}")
                    psu4 = psu[:].rearrange("p (nb q s) -> p nb q s",
                                            q=2, s=256)
                    for q in range(G0):
                        n1 = n1_0 + q
                        lhsT = TT3[0:97, 0:KT0, n1:n1 + 1]
                        nc.tensor.matmul(psu4[:, q // 2, q % 2, 0:MIXN],
                                         lhsT, ws, start=True, stop=True)
                    src = psu4[:, :, :, 0:MIXN]
                    dst = UC04[:, n1_0 // 2:(n1_0 + G0) // 2, :, :]
                    if g % 2 == 0:
                        nc.vector.tensor_copy(dst, src)
                    else:
                        nc.scalar.copy(dst, src)

                # S1 supers 0..3 cover mem rows 0:128, then interleave
                # remaining supers + S2 h=0 groups
                for s in range(4):
                    s1_super(s)
                    if s < 2:
                        pe_warm(1)
                psd_cm.__exit__(None, None, None)
                psu0_cm = tc.tile_pool(name="psU0", bufs=2, space="PSUM")
                psup0 = psu0_cm.__enter__()
                g2done = 0
                for s in range(4, NSUP):
                    s1_super(s)
                    target = (s - 3) * 8
                    while g2done < min(target, NG0):
                        s2h0_grp(g2done)
                        g2done += 1
                while g2done < NG0:
                    s2h0_grp(g2done)
                    g2done += 1
                psu0_cm.__exit__(None, None, None)
                psap_cm.__exit__(None, None, None)

                # ---- S2 h=1 + S3 per block, software-pipelined ----
                psu1_cm = tc.tile_pool(name="psU1", bufs=2, space="PSUM")
                psup1 = psu1_cm.__enter__()
                pso_cm = tc.tile_pool(name="psO", bufs=2, space="PSUM")
                psop = pso_cm.__enter__()
                obig = {}
                uc1map = {}

                def s2h1_blk(blk):
                    uc1 = ucp.tile([KT1, BLK * MIXN], F16, tag="uc1",
                                   name=f"uc1_{blk}")
                    uc14 = uc1[:].rearrange("p (nb q f) -> p nb q f",
                                            q=2, f=MIXN)
                    for half in range(2):
                        psu = psup1.tile([KT1, 2 * 512], F32, tag="psu1",
                                         name=f"psu1_{blk}_{half}")
                        psu4 = psu[:].rearrange("p (nb q s) -> p nb q s",
                                                q=2, s=256)
                        for q in range(4):
                            n1 = blk * BLK + half * 4 + q
                            lhsT = TT3[0:97, KT0:N, n1:n1 + 1]
                            nc.tensor.matmul(psu4[:, q // 2, q % 2, 0:MIXN],
                                             lhsT, ws, start=True, stop=True)
                        src = psu4[:, :, :, 0:MIXN]
                        dst = uc14[:, 2 * half:2 * half + 2, :, :]
                        nc.scalar.copy(dst, src)
                    uc1map[blk] = uc1[:].rearrange("p (n f) -> p n f", f=MIXN)

                def s3_blk(blk):
                    uc13 = uc1map.pop(blk)
                    gi = OGI[blk]
                    go = OGO[blk]
                    og = OGS[gi]
                    if go == 0:
                        for sl in range(2):
                            kp = KT0 if sl == 0 else KT1
                            obig[(gi, sl)] = obp.tile(
                                [kp, og * BLK * OUT], F16, tag=f"ob{sl}",
                                name=f"ob{sl}_{gi}")
                    for sl in range(2):
                        kp = KT0 if sl == 0 else KT1
                        pso = psop.tile([kp, BLK * OUT], F32, tag=f"pso{sl}")
                        k = 0
                        for j in (1, 2):
                            for h in range(2):
                                lt = lt_t[(j, h)]
                                lhsT = lt[:, sl * KT0:sl * KT0 + kp]
                                if h == 0:
                                    rhs = UC03[:, blk * BLK:(blk + 1) * BLK,
                                               j * OUT:(j + 1) * OUT]
                                else:
                                    rhs = uc13[:, :, j * OUT:(j + 1) * OUT]
                                nc.tensor.matmul(pso[:], lhsT, rhs,
                                                 start=(k == 0), stop=(k == 3))
                                k += 1
                        pso3 = pso[:].rearrange("p (n o) -> p n o", o=OUT)
                        if sl == 0:
                            u0 = UC03[:, blk * BLK:(blk + 1) * BLK, 0:OUT]
                        else:
                            u0 = uc13[:, :, 0:OUT]
                        ob3 = obig[(gi, sl)][:].rearrange(
                            "p (g n o) -> p g n o", g=og, o=OUT)[:, go]
                        nc.vector.tensor_add(ob3, pso3, u0)
                        if go == og - 1:
                            n1_0 = (blk - go) * BLK
                            r0 = 0 if sl == 0 else KT0
                            dst = out_d[r0:r0 + kp,
                                        n1_0:n1_0 + og * BLK, :]
                            src = obig.pop((gi, sl))[:].rearrange(
                                "p (n o) -> p n o", o=OUT)
                            nc.sync.dma_start(dst, src)

                s2h1_blk(0)
                for blk in range(1, NBLK):
                    s2h1_blk(blk)
                    s3_blk(blk - 1)
                s3_blk(NBLK - 1)
                pso_cm.__exit__(None, None, None)
                psu1_cm.__exit__(None, None, None)
    nc.compile()
    return nc


def _perm():
    # TT node-dim memory order: within a full super (8 chunks of 4 rows),
    # mem row 32s+8r+k holds logical n2 32s+4k+r; tail super (2 chunks):
    # mem 192+2r+k holds logical 192+4k+r.
    P = np.zeros(N, np.int64)
    for s in range(6):
        base = 32 * s
        for r in range(4):
            for k in range(8):
                P[base + 8 * r + k] = base + 4 * k + r
    for r in range(4):
        for k in range(2):
            P[192 + 2 * r + k] = 192 + 4 * k + r
    return P


PERM = _perm()


def _host_inputs(adj, weight, bias):
    adj = np.asarray(adj, np.float64)
    weight = np.asarray(weight, np.float64)
    bias = np.asarray(bias, np.float64)
    n = adj.shape[0]
    A = adj * (1.0 - np.eye(n))
    d0 = A.sum(0) ** -0.5
    d1 = A.sum(1) ** -0.5
    d0[np.isinf(d0)] = 0.0
    d1[np.isinf(d1)] = 0.0
    L = d0[:, None] * A * d1[None, :]
    L2 = L @ L

    p = np.array([[1.0, 0, 0], [0, 1.0, 0], [-1.0, 0, 2.0]])
    W = weight.reshape(OUT, 3, 3, C)
    Wp = np.einsum("ai,bj,oabc->ijoc", p, p, W)

    G = np.concatenate([L, L2], axis=1)
    WS = np.zeros((3 * C + 1, MIXN))
    for i in range(3):
        for j in range(3):
            WS[i * C:(i + 1) * C, j * OUT:(j + 1) * OUT] = Wp[i, j].T
    WS[96, 0:OUT] = bias
    ones = np.ones((1, n * n))
    LT1 = L.T[PERM][:, PERM]
    LT2 = L2.T[PERM][:, PERM]
    return (G.astype(np.float16), WS.astype(np.float16),
            np.ascontiguousarray(LT1).astype(np.float16),
            np.ascontiguousarray(LT2).astype(np.float16),
            ones.astype(np.float16))


def _prep_in_maps(x, adj, weight, bias):
    x = np.asarray(x)
    G, WS, LT1, LT2, ONES = _host_inputs(adj, weight, bias)
    KB = np.zeros((KT0, 1792), np.float16)
    KB[:, 0:400] = G[0:KT0]
    KB[0:KT1, 400:800] = G[KT0:N]
    KB[:, 800:1000] = LT1[0:KT0]
    KB[0:KT1, 1000:1200] = LT1[KT0:N]
    KB[:, 1200:1400] = LT2[0:KT0]
    KB[0:KT1, 1400:1600] = LT2[KT0:N]
    KB[0:97, 1600:1792] = WS
    in_maps = []
    for b in range(B):
        xb = np.asarray(x[b], np.float16)
        xt = xb.transpose(2, 1, 0)[:, PERM, :]
        # x cols ordered (n2blk, c, r): col = blk*128 + c*4 + r, n2 = 4blk+r
        xd = xb.reshape(N, N // 4, 4, C).transpose(0, 1, 3, 2)
        in_maps.append({
            "x": np.ascontiguousarray(xd.reshape(N, N * C)),
            "xt": np.ascontiguousarray(xt.reshape(C, N * N)),
            "kb": KB, "ones": ONES,
        })
    return in_maps


_PROGRAM = None


def kernel(x, adj, weight, bias):
    global _PROGRAM
    in_maps = _prep_in_maps(x, adj, weight, bias)
    if _PROGRAM is None:
        _PROGRAM = build_program()
    res = bass_utils.run_bass_kernel_spmd(_PROGRAM, in_maps,
                                          core_ids=list(range(B)))
    # device out is [n2_mem, n1, o] fp16 -> unpermute n2, transpose back
    out = np.empty((B, N, N, OUT), np.float32)
    for b in range(B):
        full = np.empty((N, N, OUT), np.float32)
        full[PERM] = res.results[b]["out"]
        out[b] = full.transpose(1, 0, 2)
    return out
